# revision 1
# baseline (speedup 1.0000x reference)
"""Cross-attention layer (vision<->text) on 8 Trainium2 NeuronCores.

Problem: B=16, Sv=St=1024, D=1024, fp32.
  q = vision @ Wq.T + bq            [B,Sv,D]
  k = text   @ Wk.T + bk            [B,St,D]
  v = text   @ Wv.T + bv            [B,St,D]
  scores = q @ k.T / sqrt(D)        [B,Sv,St]
  attn = softmax(scores, -1)
  cross_vision = attn @ v           [B,Sv,D]
  cross_text   = attn.T @ vision    [B,St,D]

Sharding: pure data-parallel over batch, 2 items per core, no collectives.

Per-core kernel design (per batch item):
  - Host pre-transposes weights: wqt = Wq.T/sqrt(D) [d,e], wkt = Wk.T, wvt = Wv.T.
    The 1/sqrt(D) is folded into wqt/bq. bv is added on the host after gather
    (attn rows sum to 1, so attn @ (v0 + bv) = attn @ v0 + bv, exact).
  - On-chip PE transposes build VT[d,s] and TT[d,t] from the natural-layout
    activations, half the seq dim at a time (SBUF economy).
  - QT[e,s] = wqt.T @ VT, KT[e,t] = wkt.T @ TT (weight tile stationary),
    Vv[t,d'] = TT.T @ wvt (TT tile stationary). All matmuls run as float32r
    (fp32 bits, PE truncates to ~fp22: full-rate at N>=512, ~2^-12 rel err).
  - S[s,t] = QT.T @ KT per 128-row s-tile; E = exp(S) straight out of PSUM on
    the ACT engine with accum_out producing row sums (scores are O(+-6), no
    max-subtraction needed for fp32 exp). rinv = 1/rowsum.
  - cross_vision s-tile: PE-transpose E row-block -> ET blocks, then
    CV = ET.T @ Vv accumulated over t-tiles, scaled by rinv at PSUM evac.
  - E is then scaled in-place by rinv (making attn rows), and
    cross_text = E.T @ V accumulated over s-tiles with raw V streamed back in.
"""

import sys

import numpy as np

if "/opt/trn_rl_repo" not in sys.path:
    sys.path.insert(0, "/opt/trn_rl_repo")

import concourse.bass as bass
import concourse.tile as tile
from concourse import bacc
from concourse import mybir

PHASE_MARKS = []  # (phase_name, first_unused_instruction_id) at each boundary

P = 128
B, SEQ, DIM = 16, 1024, 1024
N_CORES = 8
BPC = B // N_CORES  # batch items per core
NT = DIM // P  # 8 tiles of 128 along d/e
F32 = mybir.dt.float32
F32R = mybir.dt.float32r
AF = mybir.ActivationFunctionType
H = 512  # half of a seq dim / PSUM-bank-sized chunk


def _emit(tc, ident, vis, txt, wqt, wkt, wvt, bq_sb, bk_sb, cv_d, ct_d, pools, b):
    nc = tc.nc

    def mark(name):
        nid = nc._state.next_id()
        PHASE_MARKS.append((f"b{b}_{name}", nid))

    (p_act, p_kt, p_qt, p_vv, p_etb, p_wc, p_vvt, p_in, p_cvs, p_cts, p_vt,
     p_rp, p_rv, pp_t, pp_mm) = pools

    kt = p_kt.tile([P, NT, SEQ], F32R, name="kt", tag="kt")
    vv = p_vv.tile([P, NT, SEQ], F32R, name="vv", tag="vv")
    qt = p_qt.tile([P, NT, SEQ], F32R, name="qt", tag="qt")

    def prep(src_d):
        """Transpose the full [SEQ, DIM] tensor into actT[d_in, d_out, seq].

        actT shares its pool slot with e_sb (disjoint lifetimes within an
        item: actT dies after projQ, e_sb is born in phase F).
        """
        actT = p_act.tile([P, NT, SEQ], F32R, name="actT", tag="act_e")
        for l in range(NT):
            for hh in range(2):  # two [128, 512] half-row loads, dual queue
                tin = p_in.tile([P, H], F32R, name="tin", tag="xin")
                eng = nc.sync if hh == 0 else nc.scalar
                eng.dma_start(
                    out=tin,
                    in_=src_d[b, l * P:(l + 1) * P, hh * H:(hh + 1) * H].bitcast(F32R))
                tp4 = pp_t.tile([P, 4, P], F32R, name="tp4", tag="tp4")
                for j in range(4):
                    do = hh * 4 + j
                    nc.tensor.matmul(
                        tp4[:, j, :], tin[:, j * P:(j + 1) * P], ident,
                        is_transpose=True, start=(j == 0), stop=(j == 3),
                        skip_group_check=True,
                    )
                if hh == 0:
                    nc.vector.tensor_copy(actT[:, 0:4, l * P:(l + 1) * P], tp4)
                else:
                    nc.scalar.copy(actT[:, 4:8, l * P:(l + 1) * P], tp4)
        return actT

    def proj(w_d, bias_col, actT, out_sb, on_vector):
        """out_sb[e_in, eo, s] = sum_do w[do,eo].T @ actT[:, do, :] (+bias).

        One 512KB weight-column load per eo (weight read once per item),
        16 matmuls per load across the two seq halves (2 PSUM groups).
        """
        for eo in range(NT):
            wc = p_wc.tile([P, NT, P], F32R, name="wc", tag="wc")
            nc.gpsimd.dma_start(
                out=wc,
                in_=w_d[:, eo * P:(eo + 1) * P].rearrange("(do di) e -> di do e", di=P),
            )
            pss = [pp_mm.tile([P, H], F32, name=f"ps_p{i}", tag="mm") for i in range(2)]
            for do in range(NT):
                for sh in range(2):
                    nc.tensor.matmul(pss[sh], wc[:, do, :], actT[:, do, sh * H:(sh + 1) * H],
                                     start=(do == 0), stop=(do == NT - 1))
            for sh in range(2):
                dst = out_sb[:, eo, sh * H:(sh + 1) * H]
                if on_vector:
                    nc.vector.tensor_scalar_add(dst, pss[sh], scalar1=bias_col[:, eo:eo + 1])
                else:
                    nc.scalar.add(dst, pss[sh], add=bias_col[:, eo:eo + 1])

    def proj_v(actT):
        """vv[t_in, tb, d'] = (TT.T @ wvt) via VvT then PE-transpose.

        VvT[d'-block, t] is computed with the weight columns stationary (one
        512KB load per d'-block, 16 matmuls each => Wv read once per item),
        evacuated to a small staging tile, then transposed 128x128-wise into
        the Vv[t, d'] layout cross_vision needs.
        """
        for dpo in range(NT):
            wvc = p_wc.tile([P, NT, P], F32R, name="wvc", tag="wc")
            nc.gpsimd.dma_start(
                out=wvc,
                in_=wvt[:, dpo * P:(dpo + 1) * P].rearrange("(do di) e -> di do e", di=P),
            )
            pss = [pp_mm.tile([P, H], F32, name=f"ps_v{i}", tag="mm") for i in range(2)]
            for do in range(NT):
                for th in range(2):
                    nc.tensor.matmul(pss[th], wvc[:, do, :], actT[:, do, th * H:(th + 1) * H],
                                     start=(do == 0), stop=(do == NT - 1))
            vvt_tmp = p_vvt.tile([P, SEQ], F32R, name="vvt_tmp", tag="vvt")
            for th in range(2):
                nc.scalar.copy(vvt_tmp[:, th * H:(th + 1) * H], pss[th])
            for tg in range(2):
                tp4 = pp_t.tile([P, 4, P], F32R, name="tp4v", tag="tp4")
                for j in range(4):
                    tb = tg * 4 + j
                    nc.tensor.matmul(tp4[:, j, :], vvt_tmp[:, tb * P:(tb + 1) * P], ident,
                                     is_transpose=True, start=(j == 0), stop=(j == 3),
                                     skip_group_check=True)
                nc.vector.tensor_copy(vv[:, tg * 4:(tg + 1) * 4, dpo * P:(dpo + 1) * P], tp4)

    # ---- text -> TT -> KT, Vv ----
    mark("prepT")
    actT = prep(txt)
    mark("projK")
    proj(wkt, bk_sb, actT, kt, on_vector=False)
    mark("projV")
    proj_v(actT)

    # ---- vision -> VT -> QT ----
    mark("prepV")
    actV = prep(vis)
    mark("projQ")
    proj(wqt, bq_sb, actV, qt, on_vector=True)

    # ---- phase F: scores, softmax, cross_vision (per s-tile) ----
    # Software-pipelined: the scores matmuls of s-tile so+1 are emitted
    # between exp(so) (ACT) and the E-transposes that consume it, so the
    # in-order PE never waits on the ACT engine.
    mark("F")
    e_sb = p_act.tile([P, NT, SEQ], F32R, name="e_sb", tag="act_e")
    rinv = p_rv.tile([P, NT], F32, name="rinv", tag="rinv")
    rps = {}

    def scores_stile(so):
        rp = p_rp.tile([P, 2], F32, name="rp", tag="rp")
        pss = [pp_mm.tile([P, H], F32, name=f"ps_s{i}", tag="mm") for i in range(2)]
        for eo in range(NT):
            for tc_ in range(2):
                nc.tensor.matmul(pss[tc_], qt[:, eo, so * P:(so + 1) * P],
                                 kt[:, eo, tc_ * H:(tc_ + 1) * H],
                                 start=(eo == 0), stop=(eo == NT - 1))
        for tc_ in range(2):
            nc.scalar.activation(out=e_sb[:, so, tc_ * H:(tc_ + 1) * H], in_=pss[tc_],
                                 func=AF.Exp, accum_out=rp[:, tc_:tc_ + 1])
        rps[so] = rp

    scores_stile(0)
    for so in range(NT):
        if so + 1 < NT:
            scores_stile(so + 1)
        rp = rps.pop(so)
        rsum = p_rp.tile([P, 1], F32, name="rsum", tag="rsum")
        nc.vector.tensor_add(rsum, rp[:, 0:1], rp[:, 1:2])
        nc.vector.reciprocal(rinv[:, so:so + 1], rsum)

        # ET blocks for this s-tile (transpose the *unnormalized* E row-block)
        etb = p_etb.tile([P, NT, P], F32R, name="etb", tag="etb")
        for tg in range(2):
            tp4 = pp_t.tile([P, 4, P], F32R, name="tp4e", tag="tp4")
            for j in range(4):
                tt = tg * 4 + j
                nc.tensor.matmul(tp4[:, j, :], e_sb[:, so, tt * P:(tt + 1) * P], ident,
                                 is_transpose=True, start=(j == 0), stop=(j == 3),
                                 skip_group_check=True)
            nc.vector.tensor_copy(etb[:, tg * 4:(tg + 1) * 4, :], tp4)

        # normalize this E row-block in place (for cross_text later)
        nc.vector.tensor_scalar_mul(e_sb[:, so, :], e_sb[:, so, :],
                                    scalar1=rinv[:, so:so + 1])

        # cross_vision[s-tile] = rinv * (ET.T @ Vv)
        cvs = p_cvs.tile([P, DIM], F32, name="cvs", tag="cvs")
        pcv = [pp_mm.tile([P, H], F32, name=f"ps_cv{i}", tag="mm") for i in range(2)]
        for tt in range(NT):
            for dc in range(2):
                nc.tensor.matmul(pcv[dc], etb[:, tt, :], vv[:, tt, dc * H:(dc + 1) * H],
                                 start=(tt == 0), stop=(tt == NT - 1))
        for dc in range(2):
            nc.scalar.mul(cvs[:, dc * H:(dc + 1) * H], pcv[dc], mul=rinv[:, so:so + 1])
        nc.gpsimd.dma_start(out=cv_d[b, so * P:(so + 1) * P, :], in_=cvs)

    # ---- phase H: cross_text = E'.T @ V (E' already rinv-scaled) ----
    # 8 concurrent PSUM accumulation groups (6 from pmm + 2 borrowed from the
    # idle transpose pool): each V tile load feeds 8 matmuls and V is read
    # only once per d'-half. Loads alternate between the two HWDGE queues.
    mark("H")
    for dc in range(2):
        pss = [pp_mm.tile([P, H], F32, name=f"ps_ct{i}", tag="mm") for i in range(6)]
        pss += [pp_t.tile([P, H], F32, name=f"ps_ct{i + 6}", tag="tp4") for i in range(2)]
        for so in range(NT):
            vt = p_vt.tile([P, H], F32R, name="vt", tag="vt")
            eng = nc.sync if so % 2 == 0 else nc.scalar
            eng.dma_start(out=vt, in_=vis[b, so * P:(so + 1) * P, dc * H:(dc + 1) * H].bitcast(F32R))
            for tt in range(NT):
                nc.tensor.matmul(pss[tt], e_sb[:, so, tt * P:(tt + 1) * P], vt,
                                 start=(so == 0), stop=(so == NT - 1))
        for tt in range(NT):
            cts = p_cts.tile([P, H], F32, name="cts", tag="cts")
            if tt % 2 == 0:
                nc.vector.tensor_copy(cts, pss[tt])
            else:
                nc.scalar.copy(cts, pss[tt])
            nc.gpsimd.dma_start(out=ct_d[b, tt * P:(tt + 1) * P, dc * H:(dc + 1) * H],
                                  in_=cts)
    mark("end")


def build_nc():
    nc = bacc.Bacc("TRN2", target_bir_lowering=False, debug=False, num_devices=N_CORES)
    vis = nc.dram_tensor("vision", [BPC, SEQ, DIM], F32, kind="ExternalInput").ap()
    txt = nc.dram_tensor("text", [BPC, SEQ, DIM], F32, kind="ExternalInput").ap()
    wqt = nc.dram_tensor("wqt", [DIM, DIM], F32R, kind="ExternalInput").ap()
    wkt = nc.dram_tensor("wkt", [DIM, DIM], F32R, kind="ExternalInput").ap()
    wvt = nc.dram_tensor("wvt", [DIM, DIM], F32R, kind="ExternalInput").ap()
    bq_d = nc.dram_tensor("bq", [DIM], F32, kind="ExternalInput").ap()
    id_d = nc.dram_tensor("ident128", [P, P], F32R, kind="ExternalInput").ap()
    bk_d = nc.dram_tensor("bk", [DIM], F32, kind="ExternalInput").ap()
    cv_d = nc.dram_tensor("cross_vision", [BPC, SEQ, DIM], F32, kind="ExternalOutput").ap()
    ct_d = nc.dram_tensor("cross_text", [BPC, SEQ, DIM], F32, kind="ExternalOutput").ap()

    with tile.TileContext(nc) as tc:
        pools = []
        import contextlib
        with contextlib.ExitStack() as ctx:
            def sp(name, bufs):
                return ctx.enter_context(tc.tile_pool(name=name, bufs=bufs))

            p_act = sp("act", 1)
            p_kt = sp("kt", 1)
            p_qt = sp("qt", 1)
            p_vv = sp("vv", 1)
            p_etb = sp("etb", 1)
            p_wc = sp("wc", 3)
            p_vvt = sp("vvt", 2)
            p_in = sp("xin", 4)
            p_cvs = sp("cvs", 2)
            p_cts = sp("cts", 4)
            p_vt = sp("vt", 4)
            p_rp = sp("rp", 4)
            p_rv = sp("rv", 2)
            p_sm = sp("sm", 1)
            pp_t = ctx.enter_context(
                tc.tile_pool(name="pp_t", bufs=2, space=bass.MemorySpace.PSUM))
            pp_mm = ctx.enter_context(
                tc.tile_pool(name="pp_mm", bufs=6, space=bass.MemorySpace.PSUM))

            ident = p_sm.tile([P, P], F32R, name="ident")
            nc.sync.dma_start(out=ident, in_=id_d)
            bq_sb = p_sm.tile([P, NT], F32, name="bq_sb")
            nc.sync.dma_start(out=bq_sb, in_=bq_d.rearrange("(eo ei) -> ei eo", ei=P))
            bk_sb = p_sm.tile([P, NT], F32, name="bk_sb")
            nc.sync.dma_start(out=bk_sb, in_=bk_d.rearrange("(eo ei) -> ei eo", ei=P))

            pools = (p_act, p_kt, p_qt, p_vv, p_etb, p_wc, p_vvt, p_in,
                     p_cvs, p_cts, p_vt, p_rp, p_rv, pp_t, pp_mm)
            for b in range(BPC):
                _emit(tc, ident, vis, txt, wqt, wkt, wvt, bq_sb, bk_sb,
                      cv_d, ct_d, pools, b)
    nc.compile()
    return nc


_NC_CACHE = None


def _get_nc():
    global _NC_CACHE
    if _NC_CACHE is None:
        _NC_CACHE = build_nc()
    return _NC_CACHE


def make_in_maps(vision_repr, text_repr, Wq, bq, Wk, bk, Wv, bv):
    s = 1.0 / np.sqrt(np.float32(DIM))
    wqt = np.ascontiguousarray(np.asarray(Wq, np.float32).T * s)
    wkt = np.ascontiguousarray(np.asarray(Wk, np.float32).T)
    wvt = np.ascontiguousarray(np.asarray(Wv, np.float32).T)
    bq_s = np.asarray(bq, np.float32) * s
    bk_ = np.asarray(bk, np.float32)
    vis = np.asarray(vision_repr, np.float32)
    txt = np.asarray(text_repr, np.float32)
    in_maps = []
    for c in range(N_CORES):
        in_maps.append({
            "vision": vis[c * BPC:(c + 1) * BPC],
            "text": txt[c * BPC:(c + 1) * BPC],
            "wqt": wqt, "wkt": wkt, "wvt": wvt,
            "bq": bq_s, "bk": bk_,
            "ident128": np.eye(P, dtype=np.float32),
        })
    return in_maps


def kernel(vision_repr, text_repr, Wq, bq, Wk, bk, Wv, bv):
    from concourse.bass_utils import run_bass_kernel_spmd

    nc = _get_nc()
    in_maps = make_in_maps(vision_repr, text_repr, Wq, bq, Wk, bk, Wv, bv)
    res = run_bass_kernel_spmd(nc, in_maps, list(range(N_CORES))).results
    cv = np.concatenate([r_["cross_vision"] for r_ in res], axis=0)
    ct = np.concatenate([r_["cross_text"] for r_ in res], axis=0)
    cv = cv + np.asarray(bv, np.float32)[None, None, :]
    return cv, ct



# revision 10
# speedup vs baseline: 1.2867x; 1.2867x over previous
"""Cross-attention layer (vision<->text) on 8 Trainium2 NeuronCores.

Problem: B=16, Sv=St=1024, D=1024, fp32.
  q = vision @ Wq.T + bq            [B,Sv,D]
  k = text   @ Wk.T + bk            [B,St,D]
  v = text   @ Wv.T + bv            [B,St,D]
  scores = q @ k.T / sqrt(D)        [B,Sv,St]
  attn = softmax(scores, -1)
  cross_vision = attn @ v           [B,Sv,D]
  cross_text   = attn.T @ vision    [B,St,D]

Sharding: pure data-parallel over batch, 2 items per core, no collectives.

Design (v2, bf16):
  - Everything on the PE runs in bf16 (fp32 PSUM accumulation). End-to-end
    bf16 rounding measures ~6e-3 scale-rel vs the fp32 reference (gate 2e-2).
  - The PE does ONLY the six 1024^3 GEMMs per item (q/k/v projections,
    scores, attn@v, attn.T@vis): 128 matmuls each at N=512, ~216ns warm
    => ~166us/item, ~332us/core floor.
  - All transposes ride the DMA crossbar (InstDmaTransposeAnt, 2-byte dtype,
    16x128 tiles): txt^T and vis^T for the projections, E^T for cross_vision.
    No PE transpose-mode matmuls, no PSUM round-trips, no identity matrix.
  - Input casts fp32->bf16 happen inside gpsimd software-DGE DMA loads
    (the only engine that can cast in flight). Weights are pre-cast to bf16
    on the host (Wq.T pre-scaled by 1/sqrt(D)) and stay resident in SBUF.
  - vis is also kept in natural-layout bf16 (vis_n) for the cross_text GEMM,
    so phase H needs no HBM reloads.
  - Vv is produced directly in natural [t, d'] layout (TT-block stationary,
    Wv.T moving) -- no Vv transpose.
  - softmax: exp straight out of PSUM on ACT (scores are O(+-8), fp32 exp,
    no max subtraction), accum_out row sums, DVE reciprocal. E is stored
    bf16; cross_vision is scaled by rinv at PSUM evacuation (exact);
    E is then normalized in place (bf16) for cross_text.
  - Software pipelining: scores(s+2) is emitted before cv(s) so the in-order
    PE never waits on ACT/DMA; next item's txt load+transpose DMAs are
    emitted before phase H so they run under H's matmuls.
"""

import sys

import numpy as np

if "/opt/trn_rl_repo" not in sys.path:
    sys.path.insert(0, "/opt/trn_rl_repo")

import concourse.bass as bass
import concourse.tile as tile
from concourse import bacc
from concourse import mybir

PHASE_MARKS = []  # (phase_name, first_unused_instruction_id) at each boundary

P = 128
B, SEQ, DIM = 16, 1024, 1024
N_CORES = 8
BPC = B // N_CORES  # batch items per core
NT = DIM // P  # 8 tiles of 128 along d/e
F32 = mybir.dt.float32
BF = mybir.dt.bfloat16
AF = mybir.ActivationFunctionType
HH = 512  # half of a seq dim / PSUM-bank-sized chunk


class Ctx:
    pass


def _emit_prep_t(c, b):
    """Load+cast txt (gpsimd swdge) and DMA-transpose into actT_t."""
    nc = c.nc
    c.txt_n[b] = c.p_txn.tile([P, NT, SEQ], BF, name="txt_n", tag="txn")
    c.actT_t[b] = c.p_act.tile([P, NT, SEQ], BF, name="actT_t", tag="act")
    for tb in range(NT):
        nc.gpsimd.dma_start(out=c.txt_n[b][:, tb, :],
                            in_=c.txt[b, tb * P:(tb + 1) * P, :])
        nc.sync.dma_start_transpose(c.actT_t[b][:, :, tb * P:(tb + 1) * P],
                                    c.txt_n[b][:, tb, :])


def _emit_prep_v(c, b):
    """Load+cast vis into vis_n (kept for phase H) and transpose to actT_v."""
    nc = c.nc
    c.vis_n[b] = c.p_vsn.tile([P, NT, SEQ], BF, name="vis_n", tag="vsn")
    c.actT_v[b] = c.p_act.tile([P, NT, SEQ], BF, name="actT_v", tag="act")
    for sb in range(NT):
        nc.gpsimd.dma_start(out=c.vis_n[b][:, sb, :],
                            in_=c.vis[b, sb * P:(sb + 1) * P, :])
        nc.sync.dma_start_transpose(c.actT_v[b][:, :, sb * P:(sb + 1) * P],
                                    c.vis_n[b][:, sb, :])


def _emit_proj_kq(c, w_sb, bias_sb, actT, out_sb, on_vector):
    """out_sb[ei, eo, s] = sum_do w[:, eo, do, :].T @ actT[:, do, :] + bias."""
    nc = c.nc
    for eo in range(NT):
        pss = [c.pp.tile([P, HH], F32, name=f"ps_p{i}", tag="mm") for i in range(2)]
        for do in range(NT):
            for sh in range(2):
                nc.tensor.matmul(pss[sh], w_sb[:, eo, do, :],
                                 actT[:, do, sh * HH:(sh + 1) * HH],
                                 start=(do == 0), stop=(do == NT - 1))
        for sh in range(2):
            dst = out_sb[:, eo, sh * HH:(sh + 1) * HH]
            if on_vector:
                nc.vector.tensor_scalar_add(dst, pss[sh], scalar1=bias_sb[:, eo:eo + 1])
            else:
                nc.scalar.add(dst, pss[sh], add=bias_sb[:, eo:eo + 1])


def _emit_proj_v(c, b):
    """vv[ti, tb, d'] = sum_do actT_t[:, do, t-block].T @ wv[:, do, d'-half]."""
    nc = c.nc
    c.vv[b] = c.p_vv.tile([P, NT, SEQ], BF, name="vv", tag="vv")
    for tb in range(NT):
        pss = [c.pp.tile([P, HH], F32, name=f"ps_v{i}", tag="mm") for i in range(2)]
        for do in range(NT):
            for dh in range(2):
                nc.tensor.matmul(pss[dh], c.actT_t[b][:, do, tb * P:(tb + 1) * P],
                                 c.wv_sb[:, dh * (NT // 2):(dh + 1) * (NT // 2), do, :],
                                 start=(do == 0), stop=(do == NT - 1))
        for dh in range(2):
            eng = nc.vector if dh == 0 else nc.scalar
            if dh == 0:
                nc.vector.tensor_copy(c.vv[b][:, tb, dh * HH:(dh + 1) * HH], pss[dh])
            else:
                nc.scalar.copy(c.vv[b][:, tb, dh * HH:(dh + 1) * HH], pss[dh])


def _emit_f(c, b):
    """scores -> exp -> rinv -> E^T (DMA) -> cross_vision, 2-deep pipelined."""
    nc = c.nc
    e_sb = c.p_e.tile([P, NT, SEQ], BF, name="e_sb", tag="e")
    c.e_sb[b] = e_sb
    rinv = c.p_rv.tile([P, NT], F32, name="rinv", tag="rinv")
    qt, kt, vv = c.qt[b], c.kt[b], c.vv[b]
    state = {}

    def scores(so):
        pss = [c.pp.tile([P, HH], F32, name=f"ps_s{i}", tag="mm") for i in range(2)]
        for eo in range(NT):
            for th in range(2):
                nc.tensor.matmul(pss[th], qt[:, eo, so * P:(so + 1) * P],
                                 kt[:, eo, th * HH:(th + 1) * HH],
                                 start=(eo == 0), stop=(eo == NT - 1))
        rp = c.p_rp.tile([P, 2], F32, name="rp", tag="rp")
        for th in range(2):
            nc.scalar.activation(out=e_sb[:, so, th * HH:(th + 1) * HH], in_=pss[th],
                                 func=AF.Exp, accum_out=rp[:, th:th + 1])
        rsum = c.p_rp.tile([P, 1], F32, name="rsum", tag="rsum")
        nc.vector.tensor_add(rsum, rp[:, 0:1], rp[:, 1:2])
        nc.vector.reciprocal(rinv[:, so:so + 1], rsum)
        etb = c.p_etb.tile([P, NT, P], BF, name="etb", tag="etb")
        nc.sync.dma_start_transpose(etb, e_sb[:, so, :])
        # normalize E row-block in place for cross_text (after the transpose read)
        nc.vector.tensor_scalar_mul(e_sb[:, so, :], e_sb[:, so, :],
                                    scalar1=rinv[:, so:so + 1])
        state[so] = etb

    def cv(so):
        etb = state.pop(so)
        pcv = [c.pp.tile([P, HH], F32, name=f"ps_c{i}", tag="mm") for i in range(2)]
        for tt in range(NT):
            for dc in range(2):
                nc.tensor.matmul(pcv[dc], etb[:, tt, :], vv[:, tt, dc * HH:(dc + 1) * HH],
                                 start=(tt == 0), stop=(tt == NT - 1))
        cvs = c.p_cvs.tile([P, DIM], F32, name="cvs", tag="cvs")
        for dc in range(2):
            nc.scalar.mul(cvs[:, dc * HH:(dc + 1) * HH], pcv[dc], mul=rinv[:, so:so + 1])
        nc.scalar.dma_start(out=c.cv_d[b, so * P:(so + 1) * P, :], in_=cvs)

    scores(0)
    scores(1)
    for so in range(NT):
        if so + 2 < NT:
            scores(so + 2)
        cv(so)


def _emit_h(c, b):
    """cross_text[t,d] = sum_s E'[s,t] * vis[s,d] (E' normalized, all SBUF)."""
    nc = c.nc
    e_sb, vis_n = c.e_sb[b], c.vis_n[b]
    for dh in range(2):
        for tb in range(NT):
            ps = c.pp.tile([P, HH], F32, name="ps_h", tag="mm")
            for so in range(NT):
                nc.tensor.matmul(ps, e_sb[:, so, tb * P:(tb + 1) * P],
                                 vis_n[:, so, dh * HH:(dh + 1) * HH],
                                 start=(so == 0), stop=(so == NT - 1))
            cts = c.p_cts.tile([P, HH], F32, name="cts", tag="cts")
            if tb % 2 == 0:
                nc.vector.tensor_copy(cts, ps)
            else:
                nc.scalar.copy(cts, ps)
            eng = nc.sync if tb % 2 == 0 else nc.scalar
            eng.dma_start(out=c.ct_d[b, tb * P:(tb + 1) * P, dh * HH:(dh + 1) * HH],
                          in_=cts)


def build_nc():
    nc = bacc.Bacc("TRN2", target_bir_lowering=False, debug=False, num_devices=N_CORES)
    c = Ctx()
    c.nc = nc
    c.vis = nc.dram_tensor("vision", [BPC, SEQ, DIM], F32, kind="ExternalInput").ap()
    c.txt = nc.dram_tensor("text", [BPC, SEQ, DIM], F32, kind="ExternalInput").ap()
    # weights host-interleaved to [di, do, e] so DMA reads 16KB contiguous
    # per partition; biases host-interleaved to [ei, eo] (32B contiguous).
    wq_d = nc.dram_tensor("wq", [P, NT * DIM], BF, kind="ExternalInput").ap()
    wk_d = nc.dram_tensor("wk", [P, NT * DIM], BF, kind="ExternalInput").ap()
    wv_d = nc.dram_tensor("wv", [P, NT * DIM], BF, kind="ExternalInput").ap()
    bq_d = nc.dram_tensor("bq", [P, NT], F32, kind="ExternalInput").ap()
    bk_d = nc.dram_tensor("bk", [P, NT], F32, kind="ExternalInput").ap()
    c.cv_d = nc.dram_tensor("cross_vision", [BPC, SEQ, DIM], F32, kind="ExternalOutput").ap()
    c.ct_d = nc.dram_tensor("cross_text", [BPC, SEQ, DIM], F32, kind="ExternalOutput").ap()

    def mark(name):
        nid = nc._state.next_id()
        PHASE_MARKS.append((name, nid))

    with tile.TileContext(nc) as tc:
        import contextlib
        with contextlib.ExitStack() as ctx:
            def sp(name, bufs):
                return ctx.enter_context(tc.tile_pool(name=name, bufs=bufs))

            c.p_act = sp("act", 2)    # actT_t / actT_v, cycled across items
            c.p_txn = sp("txn", 1)
            c.p_vsn = sp("vsn", 1)
            c.p_kt = sp("kt", 1)
            c.p_qt = sp("qt", 1)
            c.p_vv = sp("vv", 1)
            c.p_e = sp("e", 1)
            c.p_etb = sp("etb", 2)
            c.p_cvs = sp("cvs", 2)
            c.p_cts = sp("cts", 4)
            c.p_rp = sp("rp", 4)
            c.p_rv = sp("rv", 2)
            c.p_w = sp("w", 1)
            c.pp = ctx.enter_context(
                tc.tile_pool(name="pp", bufs=8, space=bass.MemorySpace.PSUM))

            # resident weights + biases, host-interleaved to [di, eo, do, e1]
            # and loaded in per-eo 256KB chunks alternating across the two
            # hwdge queues, ordered by first use (wk -> wv -> wq) so projK's
            # first stationary block is ready in ~6us.
            c.wq_sb = c.p_w.tile([P, NT, NT, P], BF, name="wq_sb")
            c.wk_sb = c.p_w.tile([P, NT, NT, P], BF, name="wk_sb")
            c.wv_sb = c.p_w.tile([P, NT, NT, P], BF, name="wv_sb")
            c.bq_sb = c.p_w.tile([P, NT], F32, name="bq_sb")
            c.bk_sb = c.p_w.tile([P, NT], F32, name="bk_sb")
            nc.scalar.dma_start(out=c.bk_sb, in_=bk_d)
            nc.scalar.dma_start(out=c.bq_sb, in_=bq_d)
            for w_sb, w_d in ((c.wk_sb, wk_d), (c.wv_sb, wv_d), (c.wq_sb, wq_d)):
                for eo in range(NT):
                    eng = nc.sync if eo % 2 == 0 else nc.scalar
                    eng.dma_start(out=w_sb[:, eo, :, :],
                                  in_=w_d[:, eo * DIM:(eo + 1) * DIM])

            c.txt_n = {}; c.vis_n = {}; c.actT_t = {}; c.actT_v = {}
            c.qt = {}; c.kt = {}; c.vv = {}; c.e_sb = {}

            for b in range(BPC):
                if b == 0:
                    mark("b0_prep")
                    _emit_prep_t(c, 0)
                _emit_prep_v(c, b)
                mark(f"b{b}_projK")
                c.kt[b] = c.p_kt.tile([P, NT, SEQ], BF, name="kt", tag="kt")
                _emit_proj_kq(c, c.wk_sb, c.bk_sb, c.actT_t[b], c.kt[b], on_vector=False)
                mark(f"b{b}_projV")
                _emit_proj_v(c, b)
                mark(f"b{b}_projQ")
                c.qt[b] = c.p_qt.tile([P, NT, SEQ], BF, name="qt", tag="qt")
                _emit_proj_kq(c, c.wq_sb, c.bq_sb, c.actT_v[b], c.qt[b], on_vector=True)
                mark(f"b{b}_F")
                _emit_f(c, b)
                # prefetch next item's txt while H runs on the PE
                if b + 1 < BPC:
                    mark(f"b{b + 1}_prep")
                    _emit_prep_t(c, b + 1)
                mark(f"b{b}_H")
                _emit_h(c, b)
            mark("end")
    nc.compile()
    return nc


_NC_CACHE = None


def _get_nc():
    global _NC_CACHE
    if _NC_CACHE is None:
        _NC_CACHE = build_nc()
    return _NC_CACHE


def make_in_maps(vision_repr, text_repr, Wq, bq, Wk, bk, Wv, bv):
    import ml_dtypes

    def ilv(wt):  # [d, e] -> [di, (eo do e1)] so per-eo chunks are contiguous
        return np.ascontiguousarray(
            wt.reshape(NT, P, NT, P).transpose(1, 2, 0, 3).reshape(P, NT * DIM)
        ).astype(ml_dtypes.bfloat16)

    s = 1.0 / np.sqrt(np.float32(DIM))
    wq_b = ilv(np.asarray(Wq, np.float32).T * s)
    wk_b = ilv(np.asarray(Wk, np.float32).T)
    wv_b = ilv(np.asarray(Wv, np.float32).T)
    bq_s = np.ascontiguousarray((np.asarray(bq, np.float32) * s).reshape(NT, P).T)
    bk_ = np.ascontiguousarray(np.asarray(bk, np.float32).reshape(NT, P).T)
    vis = np.asarray(vision_repr, np.float32)
    txt = np.asarray(text_repr, np.float32)
    in_maps = []
    for cidx in range(N_CORES):
        in_maps.append({
            "vision": vis[cidx * BPC:(cidx + 1) * BPC],
            "text": txt[cidx * BPC:(cidx + 1) * BPC],
            "wq": wq_b, "wk": wk_b, "wv": wv_b,
            "bq": bq_s, "bk": bk_,
        })
    return in_maps


def kernel(vision_repr, text_repr, Wq, bq, Wk, bk, Wv, bv):
    from concourse.bass_utils import run_bass_kernel_spmd

    nc = _get_nc()
    in_maps = make_in_maps(vision_repr, text_repr, Wq, bq, Wk, bk, Wv, bv)
    res = run_bass_kernel_spmd(nc, in_maps, list(range(N_CORES))).results
    cv = np.concatenate([r_["cross_vision"] for r_ in res], axis=0)
    ct = np.concatenate([r_["cross_text"] for r_ in res], axis=0)
    cv = cv + np.asarray(bv, np.float32)[None, None, :]
    return cv, ct
